# revision 10
# baseline (speedup 1.0000x reference)
"""Distributed causal self-attention for 8 Trainium2 NeuronCores.

Problem: x[2,2048,1024] @ w_qkv[1024,3072] -> causal MHA (16 heads, d=64)
         -> @ w_out[1024,1024]. All fp32.

Sharding: core c (0..7) handles batch b=c//4 and head group g=c%4 (4 heads).
Each core projects qkv for its heads, runs flash attention (transposed-score
layout), then an AllToAll within each 4-core batch group converts head-
parallel attention output into token-parallel slices for the output
projection.  Core c writes output rows [b, 512*g : 512*(g+1), :].

Matmuls run in float32r (TF32-like, full PE rate); softmax in fp32.
"""

import sys

for _p in ("/opt/trn_rl_repo", "/root/.axon_site/_ro/trn_rl_repo"):
    if _p not in sys.path:
        sys.path.insert(0, _p)

import numpy as np

import concourse.bass as bass  # noqa: F401  (bass types used via tile/bacc)
import concourse.mybir as mybir
import concourse.tile as tile
from concourse import bacc
from concourse.bass_utils import run_bass_kernel_spmd

P = 128
B, T, C = 2, 2048, 1024
H, D = 16, 64
HL = 4               # heads per core
DL = HL * D          # 256 local head dims
KC = C // P          # 8 contraction tiles over C
QB = 512             # query chunk
NQ = T // QB         # 4 query chunks
NT = T // P          # 16 token tiles
G = 4                # cores per batch group
TS = T // G          # 512-token output slice per core
SCALE = 1.0 / 8.0    # 1/sqrt(64)
NEG = -1.0e30

F32 = mybir.dt.float32
F32R = mybir.dt.float32r

_CACHED = {}


def _mask_data():
    # masks[j, di*QB + i] = 0 if (di*128 + j) <= i else NEG
    m = np.full((P, 4 * QB), NEG, dtype=np.float32)
    for di in range(4):
        j = np.arange(P)[:, None]
        i = np.arange(QB)[None, :]
        m[:, di * QB:(di + 1) * QB] = np.where(di * P + j <= i, 0.0, NEG)
    return m


def _build():
    nc = bacc.Bacc("TRN2", target_bir_lowering=False, debug=False,
                   num_devices=8)

    xT = nc.dram_tensor("xT", [C, T], F32R, kind="ExternalInput")
    wq = nc.dram_tensor("wq", [C, DL], F32R, kind="ExternalInput")
    wk = nc.dram_tensor("wk", [C, DL], F32R, kind="ExternalInput")
    wv = nc.dram_tensor("wv", [C, DL], F32R, kind="ExternalInput")
    bq = nc.dram_tensor("bq", [1, DL], F32R, kind="ExternalInput")
    bk = nc.dram_tensor("bk", [1, DL], F32R, kind="ExternalInput")
    bv = nc.dram_tensor("bv", [1, DL], F32R, kind="ExternalInput")
    wo = nc.dram_tensor("wo", [DL, C], F32R, kind="ExternalInput")
    bo = nc.dram_tensor("bo", [1, C], F32R, kind="ExternalInput")
    out = nc.dram_tensor("out", [TS, C], F32, kind="ExternalOutput")

    masks_dram = nc.inline_tensor(_mask_data(), name="cmasks")

    with tile.TileContext(nc) as tc:
        with (
            tc.tile_pool(name="const", bufs=1) as cp,
            tc.tile_pool(name="persist", bufs=1) as pp,
            tc.tile_pool(name="work", bufs=3) as wk_p,
            tc.tile_pool(name="dram", bufs=1, space="DRAM") as dp,
            tc.tile_pool(name="ps_proj", bufs=3, space="PSUM") as ps_proj,
            tc.tile_pool(name="ps_sT", bufs=2, space="PSUM") as ps_sT,
            tc.tile_pool(name="ps_pv", bufs=2, space="PSUM") as ps_pv,
        ):
            # ---- constants ----
            masks = cp.tile([P, 4 * QB], F32)
            nc.sync.dma_start(masks[:], masks_dram[:])
            ones_f = cp.tile([1, QB], F32)
            nc.vector.memset(ones_f[:], 1.0)
            ones_r = cp.tile([1, QB], F32R)
            nc.vector.tensor_copy(ones_r[:], ones_f[:])
            bq_sb = cp.tile([1, DL], F32R)
            bk_sb = cp.tile([1, DL], F32R)
            bv_sb = cp.tile([1, DL], F32R)
            bo_sb = cp.tile([1, C], F32R)
            nc.sync.dma_start(bq_sb[:], bq[:])
            nc.sync.dma_start(bk_sb[:], bk[:])
            nc.sync.dma_start(bv_sb[:], bv[:])
            nc.sync.dma_start(bo_sb[:], bo[:])

            # ---- persistent activations ----
            qT_sb = pp.tile([P, 2, T], F32R)     # [d, t], d = mi*128+p
            kT_sb = pp.tile([P, 2, T], F32R)
            v_sb = pp.tile([P, NT, HL * (D + 1)], F32R)  # per head: 64 v + ones
            aoT_sb = pp.tile([P, 2, T], F32R)    # attention out^T (normalized)

            # ones columns of v_sb (softmax denominator accumulator)
            ones64 = cp.tile([P, NT * HL], F32)
            nc.vector.memset(ones64[:], 1.0)
            vones = v_sb.rearrange("p n (h e) -> p n h e", h=HL)[:, :, :, D:D + 1]
            nc.vector.tensor_copy(vones, ones64[:].rearrange(
                "p (n h) -> p n h", n=NT)[:, :, :, None])

            with tc.tile_pool(name="xw", bufs=1) as xw:
                xTr = xw.tile([P, KC, T], F32R)
                for kk in range(KC):
                    nc.sync.dma_start(
                        xTr[:, kk, :],
                        xT.rearrange("(k p) t -> k p t", p=P)[kk])
                wq_sb = xw.tile([P, KC, DL], F32R)
                wk_sb = xw.tile([P, KC, DL], F32R)
                wv_sb = xw.tile([P, KC, DL], F32R)
                nc.sync.dma_start(wq_sb[:], wq.rearrange("(k p) m -> p k m", p=P))
                nc.sync.dma_start(wk_sb[:], wk.rearrange("(k p) m -> p k m", p=P))
                nc.sync.dma_start(wv_sb[:], wv.rearrange("(k p) m -> p k m", p=P))

                # ---- phase A: qkv projection ----
                for w_sb, b_sb, dst in ((wq_sb, bq_sb, qT_sb),
                                        (wk_sb, bk_sb, kT_sb)):
                    for mi in range(2):
                        for ni in range(NQ):
                            ps = ps_proj.tile([P, QB], F32, name="proj_ps",
                                              tag="proj_ps")
                            for kk in range(KC):
                                nc.tensor.matmul(
                                    ps[:],
                                    w_sb[:, kk, mi * P:(mi + 1) * P],
                                    xTr[:, kk, ni * QB:(ni + 1) * QB],
                                    start=(kk == 0), stop=False)
                            nc.tensor.matmul(
                                ps[:], b_sb[:, mi * P:(mi + 1) * P], ones_r[:],
                                start=False, stop=True)
                            nc.scalar.copy(dst[:, mi, ni * QB:(ni + 1) * QB],
                                           ps[:])
                for ti in range(NT):
                    ps = ps_proj.tile([P, DL], F32, name="proj_ps",
                                      tag="proj_ps")
                    for kk in range(KC):
                        nc.tensor.matmul(ps[:], xTr[:, kk, ti * P:(ti + 1) * P],
                                         wv_sb[:, kk, :],
                                         start=(kk == 0), stop=False)
                    nc.tensor.matmul(ps[:], ones_r[:, :P], bv_sb[:],
                                     start=False, stop=True)
                    nc.scalar.copy(
                        v_sb.rearrange("p n (h e) -> p n h e", h=HL)
                        [:, ti, :, 0:D],
                        ps[:].rearrange("p (h e) -> p h e", e=D))
            # xw pool released; wo loads overlap attention below.

            with tc.tile_pool(name="wo_pool", bufs=1) as wop:
                wo_sb = wop.tile([P, 2, C], F32R)
                nc.sync.dma_start(wo_sb[:],
                                  wo.rearrange("(k p) n -> p k n", p=P))
                bo_bc = wop.tile([P, C], F32)
                nc.gpsimd.partition_broadcast(bo_bc[:], bo_sb[:].bitcast(F32))

                # ---- phase B: flash attention (transposed scores) ----
                for h in range(HL):
                    po = 64 * (h % 2)          # partition offset of head dims
                    mi = h // 2                # which 128-row tile
                    for qc in range(NQ):
                        nkb = 4 * qc + 4
                        pv = ps_pv.tile([P, QB], F32, name="pv_ps", tag="pv")
                        for kb in range(nkb):
                            sT = ps_sT.tile([P, QB], F32, name="sT_ps",
                                            tag="sT")
                            nc.tensor.matmul(
                                sT[:],
                                kT_sb[po:po + D, mi, kb * P:(kb + 1) * P],
                                qT_sb[po:po + D, mi, qc * QB:(qc + 1) * QB],
                                start=True, stop=True)
                            pT = wk_p.tile([P, QB], F32R, name="pT", tag="pT")
                            di = kb - 4 * qc
                            if di >= 0:
                                nc.vector.tensor_add(
                                    sT[:], sT[:],
                                    masks[:, di * QB:(di + 1) * QB])
                            nc.scalar.activation(
                                pT[:], sT[:],
                                mybir.ActivationFunctionType.Exp,
                                scale=SCALE)
                            nc.tensor.matmul(
                                pv[0:D + 1, :],
                                v_sb[:, kb, h * (D + 1):(h + 1) * (D + 1)],
                                pT[:],
                                start=(kb == 0), stop=(kb == nkb - 1))
                        rbc = wk_p.tile([D, QB], F32, name="rbc", tag="rbc",
                                        bufs=2)
                        nc.vector.reciprocal(rbc[0:1, :], pv[D:D + 1, :])
                        nc.gpsimd.partition_broadcast(rbc[:], rbc[0:1, :])
                        nc.vector.tensor_mul(
                            aoT_sb[po:po + D, mi, qc * QB:(qc + 1) * QB],
                            pv[0:D, :], rbc[:])

                # ---- phase C: partial output projection over local dims ----
                part_dram = dp.tile([T, C], F32)
                for mi2 in range(NT):
                    for ni in range(2):
                        ps = ps_proj.tile([P, QB], F32, name="proj_ps",
                                          tag="proj_ps")
                        for kk in range(2):
                            nc.tensor.matmul(
                                ps[:],
                                aoT_sb[:, kk, mi2 * P:(mi2 + 1) * P],
                                wo_sb[:, kk, ni * QB:(ni + 1) * QB],
                                start=(kk == 0), stop=(kk == 1))
                        o_sb = wk_p.tile([P, QB], F32, name="o_sb", tag="o_sb",
                                         bufs=2)
                        nc.scalar.copy(o_sb[:], ps[:])
                        nc.sync.dma_start(
                            part_dram[mi2 * P:(mi2 + 1) * P,
                                      ni * QB:(ni + 1) * QB],
                            o_sb[:])

                # ---- ReduceScatter within each 4-core batch group ----
                rs_out = dp.tile([TS, C], F32)
                nc.gpsimd.collective_compute(
                    "ReduceScatter",
                    mybir.AluOpType.add,
                    replica_groups=[[0, 1, 2, 3], [4, 5, 6, 7]],
                    ins=[part_dram[:]],
                    outs=[rs_out[:]],
                )

                # ---- phase D: add bias, write final token slice ----
                for mi2 in range(TS // P):
                    r_sb = wk_p.tile([P, C], F32, name="r_sb", tag="r_sb",
                                     bufs=2)
                    nc.sync.dma_start(
                        r_sb[:], rs_out[mi2 * P:(mi2 + 1) * P, :])
                    nc.vector.tensor_add(r_sb[:], r_sb[:], bo_bc[:])
                    nc.sync.dma_start(out[mi2 * P:(mi2 + 1) * P, :], r_sb[:])

    nc.compile()
    return nc


def kernel(x, w_qkv, b_qkv, w_out, b_out):
    x = np.ascontiguousarray(np.asarray(x, dtype=np.float32))
    w_qkv = np.asarray(w_qkv, dtype=np.float32)
    b_qkv = np.asarray(b_qkv, dtype=np.float32)
    w_out = np.ascontiguousarray(np.asarray(w_out, dtype=np.float32))
    b_out = np.asarray(b_out, dtype=np.float32)

    if "nc" not in _CACHED:
        _CACHED["nc"] = _build()
    nc = _CACHED["nc"]

    xTs = [np.ascontiguousarray(x[b_].T) for b_ in range(B)]
    bo = np.ascontiguousarray(b_out[None, :])
    in_maps = []
    for c in range(8):
        b_, g = c // 4, c % 4
        sl = slice(g * DL, (g + 1) * DL)
        in_maps.append({
            "xT": xTs[b_],
            "wq": np.ascontiguousarray(w_qkv[:, 0 * C:1 * C][:, sl]),
            "wk": np.ascontiguousarray(w_qkv[:, 1 * C:2 * C][:, sl]),
            "wv": np.ascontiguousarray(w_qkv[:, 2 * C:3 * C][:, sl]),
            "bq": np.ascontiguousarray(b_qkv[0 * C:1 * C][sl][None, :]),
            "bk": np.ascontiguousarray(b_qkv[1 * C:2 * C][sl][None, :]),
            "bv": np.ascontiguousarray(b_qkv[2 * C:3 * C][sl][None, :]),
            "wo": np.ascontiguousarray(w_out[g * DL:(g + 1) * DL, :]),
            "bo": bo,
        })
    res = run_bass_kernel_spmd(nc, in_maps, list(range(8)))
    out_full = np.empty((B, T, C), dtype=np.float32)
    for c in range(8):
        b_, g = c // 4, c % 4
        out_full[b_, g * TS:(g + 1) * TS, :] = res.results[c]["out"]
    return out_full


# revision 18
# speedup vs baseline: 1.1745x; 1.1745x over previous
"""Distributed causal self-attention for 8 Trainium2 NeuronCores.

Problem: x[2,2048,1024] @ w_qkv[1024,3072] -> causal MHA (16 heads, d=64)
         -> @ w_out[1024,1024]. All fp32.

Sharding: core c (0..7) handles batch b=c//4 and head group g=c%4 (4 heads).
Each core projects qkv for its heads, runs flash attention (transposed-score
layout), then an AllToAll within each 4-core batch group converts head-
parallel attention output into token-parallel slices for the output
projection.  Core c writes output rows [b, 512*g : 512*(g+1), :].

Matmuls run in float32r (TF32-like, full PE rate); softmax in fp32.
"""

import sys

for _p in ("/opt/trn_rl_repo", "/root/.axon_site/_ro/trn_rl_repo"):
    if _p not in sys.path:
        sys.path.insert(0, _p)

import numpy as np

import concourse.bass as bass  # noqa: F401  (bass types used via tile/bacc)
import concourse.mybir as mybir
import concourse.tile as tile
from concourse import bacc
from concourse.bass_utils import run_bass_kernel_spmd

P = 128
B, T, C = 2, 2048, 1024
H, D = 16, 64
HL = 4               # heads per core
DL = HL * D          # 256 local head dims
KC = C // P          # 8 contraction tiles over C
QB = 512             # query chunk
NQ = T // QB         # 4 query chunks
NT = T // P          # 16 token tiles
G = 4                # cores per batch group
TS = T // G          # 512-token output slice per core
SCALE = 1.0 / 8.0    # 1/sqrt(64)
NEG = -1.0e30

F32 = mybir.dt.float32
F32R = mybir.dt.float32r

_CACHED = {}


def _mask_data():
    # masks[j, di*QB + i] = 0 if (di*128 + j) <= i else NEG
    m = np.full((P, 4 * QB), NEG, dtype=np.float32)
    for di in range(4):
        j = np.arange(P)[:, None]
        i = np.arange(QB)[None, :]
        m[:, di * QB:(di + 1) * QB] = np.where(di * P + j <= i, 0.0, NEG)
    return m


def _build():
    nc = bacc.Bacc("TRN2", target_bir_lowering=False, debug=False,
                   num_devices=8)

    xT = nc.dram_tensor("xT", [C, T], F32R, kind="ExternalInput")
    wq = nc.dram_tensor("wq", [C, DL], F32R, kind="ExternalInput")
    wk = nc.dram_tensor("wk", [C, DL], F32R, kind="ExternalInput")
    wv = nc.dram_tensor("wv", [C, DL], F32R, kind="ExternalInput")
    bq = nc.dram_tensor("bq", [1, DL], F32R, kind="ExternalInput")
    bk = nc.dram_tensor("bk", [1, DL], F32R, kind="ExternalInput")
    bv = nc.dram_tensor("bv", [1, DL], F32R, kind="ExternalInput")
    wo = nc.dram_tensor("wo", [DL, C], F32R, kind="ExternalInput")
    bo = nc.dram_tensor("bo", [1, C], F32R, kind="ExternalInput")
    # per query-chunk ReduceScatter slices: rows qc*512 + g*128 .. +128
    out = nc.dram_tensor("out", [NQ, P, C], F32, kind="ExternalOutput")

    masks_dram = nc.inline_tensor(_mask_data(), name="cmasks")

    with tile.TileContext(nc) as tc:
        with (
            tc.tile_pool(name="const", bufs=1) as cp,
            tc.tile_pool(name="persist", bufs=1) as pp,
            tc.tile_pool(name="work", bufs=3) as wk_p,
            tc.tile_pool(name="dram", bufs=1, space="DRAM") as dp,
            tc.tile_pool(name="ps_proj", bufs=2, space="PSUM") as ps_proj,
            tc.tile_pool(name="ps_sT", bufs=2, space="PSUM") as ps_sT,
            tc.tile_pool(name="ps_pv", bufs=2, space="PSUM") as ps_pv,
        ):
            # ---- constants ----
            masks = cp.tile([P, 4 * QB], F32)
            nc.sync.dma_start(masks[:], masks_dram[:])
            ones_f = cp.tile([1, QB], F32)
            nc.vector.memset(ones_f[:], 1.0)
            ones_r = cp.tile([1, QB], F32R)
            nc.vector.tensor_copy(ones_r[:], ones_f[:])
            bq_sb = cp.tile([1, DL], F32R)
            bk_sb = cp.tile([1, DL], F32R)
            bv_sb = cp.tile([1, DL], F32R)
            bo_sb = cp.tile([1, C], F32R)
            nc.sync.dma_start(bq_sb[:], bq[:])
            nc.sync.dma_start(bk_sb[:], bk[:])
            nc.sync.dma_start(bv_sb[:], bv[:])
            nc.sync.dma_start(bo_sb[:], bo[:])

            # ---- persistent activations ----
            qT_sb = pp.tile([P, 2, T], F32R)     # [d, t], d = mi*128+p
            kT_sb = pp.tile([P, 2, T], F32R)
            v_sb = pp.tile([P, NT, HL * (D + 1)], F32R)  # per head: 64 v + ones
            aoT_sb = pp.tile([P, 2, T], F32R)    # attention out^T (normalized)

            # ones columns of v_sb (softmax denominator accumulator)
            ones64 = cp.tile([P, NT * HL], F32)
            nc.vector.memset(ones64[:], 1.0)
            vones = v_sb.rearrange("p n (h e) -> p n h e", h=HL)[:, :, :, D:D + 1]
            nc.vector.tensor_copy(vones, ones64[:].rearrange(
                "p (n h) -> p n h", n=NT)[:, :, :, None])

            with tc.tile_pool(name="xw", bufs=1) as xw:
                xTr = xw.tile([P, KC, T], F32R)
                for kk in range(KC):
                    nc.sync.dma_start(
                        xTr[:, kk, :],
                        xT.rearrange("(k p) t -> k p t", p=P)[kk])
                wq_sb = xw.tile([P, KC, DL], F32R)
                wk_sb = xw.tile([P, KC, DL], F32R)
                wv_sb = xw.tile([P, KC, DL], F32R)
                nc.sync.dma_start(wq_sb[:], wq.rearrange("(k p) m -> p k m", p=P))
                nc.sync.dma_start(wk_sb[:], wk.rearrange("(k p) m -> p k m", p=P))
                nc.sync.dma_start(wv_sb[:], wv.rearrange("(k p) m -> p k m", p=P))

                # ---- phase A: qkv projection ----
                for w_sb, b_sb, dst in ((wq_sb, bq_sb, qT_sb),
                                        (wk_sb, bk_sb, kT_sb)):
                    for mi in range(2):
                        for ni in range(NQ):
                            ps = ps_proj.tile([P, QB], F32, name="proj_ps",
                                              tag="proj_ps")
                            for kk in range(KC):
                                nc.tensor.matmul(
                                    ps[:],
                                    w_sb[:, kk, mi * P:(mi + 1) * P],
                                    xTr[:, kk, ni * QB:(ni + 1) * QB],
                                    start=(kk == 0), stop=False)
                            nc.tensor.matmul(
                                ps[:], b_sb[:, mi * P:(mi + 1) * P], ones_r[:],
                                start=False, stop=True)
                            nc.vector.tensor_copy(
                                dst[:, mi, ni * QB:(ni + 1) * QB], ps[:])
                for ti in range(NT):
                    ps = ps_proj.tile([P, DL], F32, name="proj_ps",
                                      tag="proj_ps")
                    for kk in range(KC):
                        nc.tensor.matmul(ps[:], xTr[:, kk, ti * P:(ti + 1) * P],
                                         wv_sb[:, kk, :],
                                         start=(kk == 0), stop=False)
                    nc.tensor.matmul(ps[:], ones_r[:, :P], bv_sb[:],
                                     start=False, stop=True)
                    nc.vector.tensor_copy(
                        v_sb.rearrange("p n (h e) -> p n h e", h=HL)
                        [:, ti, :, 0:D],
                        ps[:].rearrange("p (h e) -> p h e", e=D))
            # xw pool released; wo loads overlap attention below.

            with tc.tile_pool(name="wo_pool", bufs=1) as wop:
                wo_sb = wop.tile([P, 2, C], F32R)
                nc.sync.dma_start(wo_sb[:],
                                  wo.rearrange("(k p) n -> p k n", p=P))
                bo_bc = wop.tile([P, C], F32)
                nc.gpsimd.partition_broadcast(bo_bc[:], bo_sb[:].bitcast(F32))

                # ---- phases B+C interleaved per query chunk ----
                part_dram = dp.tile([T, C], F32)
                rs_out = dp.tile([NQ, P, C], F32)
                for qc in range(NQ):
                    # flash attention for this query chunk, all local heads
                    nkb = 4 * qc + 4
                    for h in range(HL):
                        po = 64 * (h % 2)      # partition offset of head dims
                        mi = h // 2            # which 128-row tile
                        pv = ps_pv.tile([P, QB], F32, name="pv_ps", tag="pv")
                        for kp in range(nkb // 2):     # key-block pairs
                            sT = ps_sT.tile([P, 2 * QB], F32, name="sT_ps",
                                            tag="sT")
                            for half in range(2):
                                kb = 2 * kp + half
                                nc.tensor.matmul(
                                    sT[:, half * QB:(half + 1) * QB],
                                    kT_sb[po:po + D, mi, kb * P:(kb + 1) * P],
                                    qT_sb[po:po + D, mi,
                                          qc * QB:(qc + 1) * QB],
                                    start=True, stop=True)
                            if 2 * kp >= 4 * qc:       # diagonal pair
                                di = 2 * kp - 4 * qc
                                nc.vector.tensor_add(
                                    sT[:], sT[:],
                                    masks[:, di * QB:(di + 2) * QB])
                            pT = wk_p.tile([P, 2 * QB], F32R, name="pT",
                                           tag="pT")
                            nc.scalar.activation(
                                pT[:], sT[:],
                                mybir.ActivationFunctionType.Exp,
                                scale=SCALE)
                            for half in range(2):
                                kb = 2 * kp + half
                                nc.tensor.matmul(
                                    pv[0:D + 1, :],
                                    v_sb[:, kb,
                                         h * (D + 1):(h + 1) * (D + 1)],
                                    pT[:, half * QB:(half + 1) * QB],
                                    start=(kb == 0), stop=(kb == nkb - 1))
                        rbc = wk_p.tile([D, QB], F32, name="rbc", tag="rbc",
                                        bufs=2)
                        nc.vector.reciprocal(rbc[0:1, :], pv[D:D + 1, :])
                        nc.gpsimd.partition_broadcast(rbc[:], rbc[0:1, :])
                        nc.vector.tensor_mul(
                            aoT_sb[po:po + D, mi, qc * QB:(qc + 1) * QB],
                            pv[0:D, :], rbc[:])

                    # partial output projection for this chunk's tokens
                    for mi2 in range(4 * qc, 4 * qc + 4):
                        for ni in range(2):
                            ps = ps_proj.tile([P, QB], F32, name="proj_ps",
                                              tag="proj_ps")
                            for kk in range(2):
                                nc.tensor.matmul(
                                    ps[:],
                                    aoT_sb[:, kk, mi2 * P:(mi2 + 1) * P],
                                    wo_sb[:, kk, ni * QB:(ni + 1) * QB],
                                    start=(kk == 0), stop=(kk == 1))
                            o_sb = wk_p.tile([P, QB], F32, name="o_sb",
                                             tag="o_sb", bufs=2)
                            nc.scalar.copy(o_sb[:], ps[:])
                            nc.sync.dma_start(
                                part_dram[mi2 * P:(mi2 + 1) * P,
                                          ni * QB:(ni + 1) * QB],
                                o_sb[:])

                    # chunked ReduceScatter overlaps later chunks' attention
                    nc.gpsimd.collective_compute(
                        "ReduceScatter",
                        mybir.AluOpType.add,
                        replica_groups=[[0, 1, 2, 3], [4, 5, 6, 7]],
                        ins=[part_dram[qc * QB:(qc + 1) * QB, :]],
                        outs=[rs_out[qc]],
                    )
                    r_sb = wk_p.tile([P, C], F32, name="r_sb", tag="r_sb",
                                     bufs=2)
                    nc.sync.dma_start(r_sb[:], rs_out[qc])
                    nc.vector.tensor_add(r_sb[:], r_sb[:], bo_bc[:])
                    nc.sync.dma_start(out[qc], r_sb[:])

    nc.compile()
    return nc


def kernel(x, w_qkv, b_qkv, w_out, b_out):
    x = np.ascontiguousarray(np.asarray(x, dtype=np.float32))
    w_qkv = np.asarray(w_qkv, dtype=np.float32)
    b_qkv = np.asarray(b_qkv, dtype=np.float32)
    w_out = np.ascontiguousarray(np.asarray(w_out, dtype=np.float32))
    b_out = np.asarray(b_out, dtype=np.float32)

    if "nc" not in _CACHED:
        _CACHED["nc"] = _build()
    nc = _CACHED["nc"]

    xTs = [np.ascontiguousarray(x[b_].T) for b_ in range(B)]
    bo = np.ascontiguousarray(b_out[None, :])
    in_maps = []
    for c in range(8):
        b_, g = c // 4, c % 4
        sl = slice(g * DL, (g + 1) * DL)
        in_maps.append({
            "xT": xTs[b_],
            "wq": np.ascontiguousarray(w_qkv[:, 0 * C:1 * C][:, sl]),
            "wk": np.ascontiguousarray(w_qkv[:, 1 * C:2 * C][:, sl]),
            "wv": np.ascontiguousarray(w_qkv[:, 2 * C:3 * C][:, sl]),
            "bq": np.ascontiguousarray(b_qkv[0 * C:1 * C][sl][None, :]),
            "bk": np.ascontiguousarray(b_qkv[1 * C:2 * C][sl][None, :]),
            "bv": np.ascontiguousarray(b_qkv[2 * C:3 * C][sl][None, :]),
            "wo": np.ascontiguousarray(w_out[g * DL:(g + 1) * DL, :]),
            "bo": bo,
        })
    res = run_bass_kernel_spmd(nc, in_maps, list(range(8)))
    out_full = np.empty((B, T, C), dtype=np.float32)
    for c in range(8):
        b_, g = c // 4, c % 4
        o = res.results[c]["out"]          # [NQ, P, C]
        for qc in range(NQ):
            r0 = qc * QB + g * P
            out_full[b_, r0:r0 + P, :] = o[qc]
    return out_full


# revision 19
# speedup vs baseline: 1.3263x; 1.1293x over previous
"""Distributed causal self-attention for 8 Trainium2 NeuronCores.

Problem: x[2,2048,1024] @ w_qkv[1024,3072] -> causal MHA (16 heads, d=64)
         -> @ w_out[1024,1024]. All fp32.

Sharding: core c (0..7) handles batch b=c//4 and head group g=c%4 (4 heads).
Each core projects qkv for its heads, runs flash attention (transposed-score
layout), then an AllToAll within each 4-core batch group converts head-
parallel attention output into token-parallel slices for the output
projection.  Core c writes output rows [b, 512*g : 512*(g+1), :].

Matmuls run in float32r (TF32-like, full PE rate); softmax in fp32.
"""

import sys

for _p in ("/opt/trn_rl_repo", "/root/.axon_site/_ro/trn_rl_repo"):
    if _p not in sys.path:
        sys.path.insert(0, _p)

import numpy as np

import concourse.bass as bass  # noqa: F401  (bass types used via tile/bacc)
import concourse.mybir as mybir
import concourse.tile as tile
from concourse import bacc
from concourse.bass_utils import run_bass_kernel_spmd

P = 128
B, T, C = 2, 2048, 1024
H, D = 16, 64
HL = 4               # heads per core
DL = HL * D          # 256 local head dims
KC = C // P          # 8 contraction tiles over C
QB = 512             # query chunk
NQ = T // QB         # 4 query chunks
NT = T // P          # 16 token tiles
G = 4                # cores per batch group
TS = T // G          # 512-token output slice per core
SCALE = 1.0 / 8.0    # 1/sqrt(64)
NEG = -1.0e30

F32 = mybir.dt.float32
F32R = mybir.dt.float32r

_CACHED = {}


def _mask_data():
    # masks[j, di*QB + i] = 0 if (di*128 + j) <= i else NEG
    m = np.full((P, 4 * QB), NEG, dtype=np.float32)
    for di in range(4):
        j = np.arange(P)[:, None]
        i = np.arange(QB)[None, :]
        m[:, di * QB:(di + 1) * QB] = np.where(di * P + j <= i, 0.0, NEG)
    return m


def _build():
    nc = bacc.Bacc("TRN2", target_bir_lowering=False, debug=False,
                   num_devices=8)

    xT = nc.dram_tensor("xT", [C, T], F32R, kind="ExternalInput")
    wq = nc.dram_tensor("wq", [C, DL], F32R, kind="ExternalInput")
    wk = nc.dram_tensor("wk", [C, DL], F32R, kind="ExternalInput")
    wv = nc.dram_tensor("wv", [C, DL], F32R, kind="ExternalInput")
    bq = nc.dram_tensor("bq", [1, DL], F32R, kind="ExternalInput")
    bk = nc.dram_tensor("bk", [1, DL], F32R, kind="ExternalInput")
    bv = nc.dram_tensor("bv", [1, DL], F32R, kind="ExternalInput")
    wo = nc.dram_tensor("wo", [DL, C], F32R, kind="ExternalInput")
    bo = nc.dram_tensor("bo", [1, C], F32R, kind="ExternalInput")
    # per query-chunk ReduceScatter slices: rows qc*512 + g*128 .. +128
    out = nc.dram_tensor("out", [NQ, P, C], F32, kind="ExternalOutput")

    masks_dram = nc.inline_tensor(_mask_data(), name="cmasks")

    with tile.TileContext(nc) as tc:
        with (
            tc.tile_pool(name="const", bufs=1) as cp,
            tc.tile_pool(name="persist", bufs=1) as pp,
            tc.tile_pool(name="work", bufs=3) as wk_p,
            tc.tile_pool(name="dram", bufs=1, space="DRAM") as dp,
            tc.tile_pool(name="ps_proj", bufs=2, space="PSUM") as ps_proj,
            tc.tile_pool(name="ps_sT", bufs=2, space="PSUM") as ps_sT,
            tc.tile_pool(name="ps_pv", bufs=2, space="PSUM") as ps_pv,
        ):
            # ---- constants ----
            masks = cp.tile([P, 4 * QB], F32)
            nc.sync.dma_start(masks[:], masks_dram[:])
            ones_f = cp.tile([1, QB], F32)
            nc.vector.memset(ones_f[:], 1.0)
            ones_r = cp.tile([1, QB], F32R)
            nc.vector.tensor_copy(ones_r[:], ones_f[:])
            bq_sb = cp.tile([1, DL], F32R)
            bk_sb = cp.tile([1, DL], F32R)
            bv_sb = cp.tile([1, DL], F32R)
            bo_sb = cp.tile([1, C], F32R)
            nc.sync.dma_start(bq_sb[:], bq[:])
            nc.sync.dma_start(bk_sb[:], bk[:])
            nc.sync.dma_start(bv_sb[:], bv[:])
            nc.sync.dma_start(bo_sb[:], bo[:])

            # ---- persistent activations ----
            qT_sb = pp.tile([P, 2, T], F32R)     # [d, t], d = mi*128+p
            kT_sb = pp.tile([P, 2, T], F32R)
            v_sb = pp.tile([P, NT, HL * (D + 1)], F32R)  # per head: 64 v + ones
            aoT_sb = pp.tile([P, 2, T], F32R)    # attention out^T (normalized)

            # ones columns of v_sb (softmax denominator accumulator)
            ones64 = cp.tile([P, NT * HL], F32)
            nc.vector.memset(ones64[:], 1.0)
            vones = v_sb.rearrange("p n (h e) -> p n h e", h=HL)[:, :, :, D:D + 1]
            nc.vector.tensor_copy(vones, ones64[:].rearrange(
                "p (n h) -> p n h", n=NT)[:, :, :, None])

            with tc.tile_pool(name="xw", bufs=1) as xw:
                xTr = xw.tile([P, KC, T], F32R)
                for kk in range(KC):
                    nc.sync.dma_start(
                        xTr[:, kk, :],
                        xT.rearrange("(k p) t -> k p t", p=P)[kk])
                wq_sb = xw.tile([P, KC, DL], F32R)
                wk_sb = xw.tile([P, KC, DL], F32R)
                wv_sb = xw.tile([P, KC, DL], F32R)
                nc.sync.dma_start(wq_sb[:], wq.rearrange("(k p) m -> p k m", p=P))
                nc.sync.dma_start(wk_sb[:], wk.rearrange("(k p) m -> p k m", p=P))
                nc.sync.dma_start(wv_sb[:], wv.rearrange("(k p) m -> p k m", p=P))

                # ---- phase A: qkv projection ----
                for w_sb, b_sb, dst in ((wq_sb, bq_sb, qT_sb),
                                        (wk_sb, bk_sb, kT_sb)):
                    for mi in range(2):
                        for ni in range(NQ):
                            ps = ps_proj.tile([P, QB], F32, name="proj_ps",
                                              tag="proj_ps")
                            for kk in range(KC):
                                nc.tensor.matmul(
                                    ps[:],
                                    w_sb[:, kk, mi * P:(mi + 1) * P],
                                    xTr[:, kk, ni * QB:(ni + 1) * QB],
                                    start=(kk == 0), stop=False)
                            nc.tensor.matmul(
                                ps[:], b_sb[:, mi * P:(mi + 1) * P], ones_r[:],
                                start=False, stop=True)
                            nc.vector.tensor_copy(
                                dst[:, mi, ni * QB:(ni + 1) * QB], ps[:])
                for ti in range(NT):
                    ps = ps_proj.tile([P, DL], F32, name="proj_ps",
                                      tag="proj_ps")
                    for kk in range(KC):
                        nc.tensor.matmul(ps[:], xTr[:, kk, ti * P:(ti + 1) * P],
                                         wv_sb[:, kk, :],
                                         start=(kk == 0), stop=False)
                    nc.tensor.matmul(ps[:], ones_r[:, :P], bv_sb[:],
                                     start=False, stop=True)
                    nc.vector.tensor_copy(
                        v_sb.rearrange("p n (h e) -> p n h e", h=HL)
                        [:, ti, :, 0:D],
                        ps[:].rearrange("p (h e) -> p h e", e=D))
            # xw pool released; wo loads overlap attention below.

            with tc.tile_pool(name="wo_pool", bufs=1) as wop:
                wo_sb = wop.tile([P, 2, C], F32R)
                nc.sync.dma_start(wo_sb[:],
                                  wo.rearrange("(k p) n -> p k n", p=P))
                bo_bc = wop.tile([P, C], F32)
                nc.gpsimd.partition_broadcast(bo_bc[:], bo_sb[:].bitcast(F32))

                # ---- phases B+C interleaved per query chunk ----
                BF16 = mybir.dt.bfloat16
                part_dram = dp.tile([T, C], BF16)
                rs_out = dp.tile([NQ, P, C], BF16)
                for qc in range(NQ):
                    # flash attention: head pairs interleaved at the
                    # key-block level so PE stays busy while ACT runs exp
                    nkb = 4 * qc + 4
                    for hp in range(HL // 2):
                        heads = (2 * hp, 2 * hp + 1)
                        pvs = {}
                        for h in heads:
                            pvs[h] = ps_pv.tile([P, QB], F32,
                                                name="pv_ps", tag="pv")
                        for kp in range(nkb // 2):     # key-block pairs
                            for h in heads:
                                po = 64 * (h % 2)
                                mi = h // 2
                                sT = ps_sT.tile([P, 2 * QB], F32,
                                                name="sT_ps", tag="sT")
                                for half in range(2):
                                    kb = 2 * kp + half
                                    nc.tensor.matmul(
                                        sT[:, half * QB:(half + 1) * QB],
                                        kT_sb[po:po + D, mi,
                                              kb * P:(kb + 1) * P],
                                        qT_sb[po:po + D, mi,
                                              qc * QB:(qc + 1) * QB],
                                        start=True, stop=True)
                                if 2 * kp >= 4 * qc:   # diagonal pair
                                    di = 2 * kp - 4 * qc
                                    nc.vector.tensor_add(
                                        sT[:], sT[:],
                                        masks[:, di * QB:(di + 2) * QB])
                                pT = wk_p.tile([P, 2 * QB], F32R, name="pT",
                                               tag="pT")
                                nc.scalar.activation(
                                    pT[:], sT[:],
                                    mybir.ActivationFunctionType.Exp,
                                    scale=SCALE)
                                for half in range(2):
                                    kb = 2 * kp + half
                                    nc.tensor.matmul(
                                        pvs[h][0:D + 1, :],
                                        v_sb[:, kb,
                                             h * (D + 1):(h + 1) * (D + 1)],
                                        pT[:, half * QB:(half + 1) * QB],
                                        start=(kb == 0),
                                        stop=(kb == nkb - 1))
                        for h in heads:
                            po = 64 * (h % 2)
                            mi = h // 2
                            rbc = wk_p.tile([D, QB], F32, name="rbc",
                                            tag="rbc", bufs=2)
                            nc.vector.reciprocal(rbc[0:1, :],
                                                 pvs[h][D:D + 1, :])
                            nc.gpsimd.partition_broadcast(rbc[:], rbc[0:1, :])
                            nc.vector.tensor_mul(
                                aoT_sb[po:po + D, mi, qc * QB:(qc + 1) * QB],
                                pvs[h][0:D, :], rbc[:])

                    # partial output projection for this chunk's tokens
                    for mi2 in range(4 * qc, 4 * qc + 4):
                        for ni in range(2):
                            ps = ps_proj.tile([P, QB], F32, name="proj_ps",
                                              tag="proj_ps")
                            for kk in range(2):
                                nc.tensor.matmul(
                                    ps[:],
                                    aoT_sb[:, kk, mi2 * P:(mi2 + 1) * P],
                                    wo_sb[:, kk, ni * QB:(ni + 1) * QB],
                                    start=(kk == 0), stop=(kk == 1))
                            o_sb = wk_p.tile([P, QB], BF16, name="o_sb",
                                             tag="o_sb", bufs=2)
                            nc.scalar.copy(o_sb[:], ps[:])
                            nc.sync.dma_start(
                                part_dram[mi2 * P:(mi2 + 1) * P,
                                          ni * QB:(ni + 1) * QB],
                                o_sb[:])

                    # chunked ReduceScatter overlaps later chunks' attention
                    nc.gpsimd.collective_compute(
                        "ReduceScatter",
                        mybir.AluOpType.add,
                        replica_groups=[[0, 1, 2, 3], [4, 5, 6, 7]],
                        ins=[part_dram[qc * QB:(qc + 1) * QB, :]],
                        outs=[rs_out[qc]],
                    )
                    r_sb = wk_p.tile([P, C], BF16, name="r_sb", tag="r_sb",
                                     bufs=2)
                    nc.sync.dma_start(r_sb[:], rs_out[qc])
                    f_sb = wk_p.tile([P, C], F32, name="f_sb", tag="f_sb",
                                     bufs=2)
                    nc.vector.tensor_add(f_sb[:], r_sb[:], bo_bc[:])
                    nc.sync.dma_start(out[qc], f_sb[:])

    nc.compile()
    return nc


def kernel(x, w_qkv, b_qkv, w_out, b_out):
    x = np.ascontiguousarray(np.asarray(x, dtype=np.float32))
    w_qkv = np.asarray(w_qkv, dtype=np.float32)
    b_qkv = np.asarray(b_qkv, dtype=np.float32)
    w_out = np.ascontiguousarray(np.asarray(w_out, dtype=np.float32))
    b_out = np.asarray(b_out, dtype=np.float32)

    if "nc" not in _CACHED:
        _CACHED["nc"] = _build()
    nc = _CACHED["nc"]

    xTs = [np.ascontiguousarray(x[b_].T) for b_ in range(B)]
    bo = np.ascontiguousarray(b_out[None, :])
    in_maps = []
    for c in range(8):
        b_, g = c // 4, c % 4
        sl = slice(g * DL, (g + 1) * DL)
        in_maps.append({
            "xT": xTs[b_],
            "wq": np.ascontiguousarray(w_qkv[:, 0 * C:1 * C][:, sl]),
            "wk": np.ascontiguousarray(w_qkv[:, 1 * C:2 * C][:, sl]),
            "wv": np.ascontiguousarray(w_qkv[:, 2 * C:3 * C][:, sl]),
            "bq": np.ascontiguousarray(b_qkv[0 * C:1 * C][sl][None, :]),
            "bk": np.ascontiguousarray(b_qkv[1 * C:2 * C][sl][None, :]),
            "bv": np.ascontiguousarray(b_qkv[2 * C:3 * C][sl][None, :]),
            "wo": np.ascontiguousarray(w_out[g * DL:(g + 1) * DL, :]),
            "bo": bo,
        })
    res = run_bass_kernel_spmd(nc, in_maps, list(range(8)))
    out_full = np.empty((B, T, C), dtype=np.float32)
    for c in range(8):
        b_, g = c // 4, c % 4
        o = res.results[c]["out"]          # [NQ, P, C]
        for qc in range(NQ):
            r0 = qc * QB + g * P
            out_full[b_, r0:r0 + P, :] = o[qc]
    return out_full


# revision 26
# speedup vs baseline: 1.4949x; 1.1271x over previous
"""Distributed causal self-attention for 8 Trainium2 NeuronCores.

Problem: x[2,2048,1024] @ w_qkv[1024,3072] -> causal MHA (16 heads, d=64)
         -> @ w_out[1024,1024]. All fp32.

Sharding: core c (0..7) handles batch b=c//4 and head group g=c%4 (4 heads).
Each core projects qkv for its heads, runs flash attention (transposed-score
layout), then an AllToAll within each 4-core batch group converts head-
parallel attention output into token-parallel slices for the output
projection.  Core c writes output rows [b, 512*g : 512*(g+1), :].

Matmuls run in float32r (TF32-like, full PE rate); softmax in fp32.
"""

import sys

for _p in ("/opt/trn_rl_repo", "/root/.axon_site/_ro/trn_rl_repo"):
    if _p not in sys.path:
        sys.path.insert(0, _p)

import numpy as np

import concourse.bass as bass  # noqa: F401  (bass types used via tile/bacc)
import concourse.mybir as mybir
import concourse.tile as tile
from concourse import bacc
from concourse.bass_utils import run_bass_kernel_spmd

P = 128
B, T, C = 2, 2048, 1024
H, D = 16, 64
HL = 4               # heads per core
DL = HL * D          # 256 local head dims
KC = C // P          # 8 contraction tiles over C
QB = 512             # query chunk
NQ = T // QB         # 4 query chunks
NT = T // P          # 16 token tiles
G = 4                # cores per batch group
TS = T // G          # 512-token output slice per core
SCALE = 1.0 / 8.0    # 1/sqrt(64)
NEG = -1.0e30

F32 = mybir.dt.float32
F32R = mybir.dt.float32r

_CACHED = {}


def _mask_data():
    # tril mask: 0 where key j <= query i, NEG above the diagonal
    j = np.arange(P)[:, None]
    i = np.arange(P)[None, :]
    return np.where(j <= i, 0.0, NEG).astype(np.float32)


def _build():
    nc = bacc.Bacc("TRN2", target_bir_lowering=False, debug=False,
                   num_devices=8)

    xT = nc.dram_tensor("xT", [C, T], F32R, kind="ExternalInput")
    wq = nc.dram_tensor("wq", [C, DL], F32R, kind="ExternalInput")
    wk = nc.dram_tensor("wk", [C, DL], F32R, kind="ExternalInput")
    wv = nc.dram_tensor("wv", [C, DL], F32R, kind="ExternalInput")
    bq = nc.dram_tensor("bq", [1, DL], F32R, kind="ExternalInput")
    bk = nc.dram_tensor("bk", [1, DL], F32R, kind="ExternalInput")
    bv = nc.dram_tensor("bv", [1, DL], F32R, kind="ExternalInput")
    wo = nc.dram_tensor("wo", [DL, C], F32R, kind="ExternalInput")
    bo = nc.dram_tensor("bo", [1, C], F32R, kind="ExternalInput")
    # per query-chunk ReduceScatter slices: rows qc*512 + g*128 .. +128
    out = nc.dram_tensor("out", [NQ, P, C], F32, kind="ExternalOutput")

    masks_dram = nc.inline_tensor(_mask_data(), name="cmasks")

    with tile.TileContext(nc) as tc:
        with (
            tc.tile_pool(name="const", bufs=1) as cp,
            tc.tile_pool(name="persist", bufs=1) as pp,
            tc.tile_pool(name="work", bufs=3) as wk_p,
            tc.tile_pool(name="dram", bufs=1, space="DRAM") as dp,
            tc.tile_pool(name="ps_proj", bufs=2, space="PSUM") as ps_proj,
            tc.tile_pool(name="ps_sT", bufs=2, space="PSUM") as ps_sT,
            tc.tile_pool(name="ps_pv", bufs=2, space="PSUM") as ps_pv,
        ):
            # ---- constants ----
            masks = cp.tile([P, P], F32)
            nc.sync.dma_start(masks[:], masks_dram[:])
            ones_f = cp.tile([1, QB], F32)
            nc.vector.memset(ones_f[:], 1.0)
            ones_r = cp.tile([1, QB], F32R)
            nc.vector.tensor_copy(ones_r[:], ones_f[:])
            # q/k biases as per-partition columns [128, 2] (mi-major)
            bq_col = cp.tile([P, 2], F32)
            bk_col = cp.tile([P, 2], F32)
            nc.sync.dma_start(
                bq_col[:], bq.bitcast(F32)[0, :].rearrange("(m p) -> p m", p=P))
            nc.sync.dma_start(
                bk_col[:], bk.bitcast(F32)[0, :].rearrange("(m p) -> p m", p=P))
            bv_sb = cp.tile([1, DL], F32R)
            bo_sb = cp.tile([1, C], F32R)
            nc.sync.dma_start(bv_sb[:], bv[:])
            nc.sync.dma_start(bo_sb[:], bo[:])

            # ---- persistent activations ----
            qT_sb = pp.tile([P, 2, T], F32R)     # [d, t], d = mi*128+p
            kT_sb = pp.tile([P, 2, T], F32R)
            v_sb = pp.tile([P, NT, HL * (D + 1)], F32R)  # per head: 64 v + ones
            aoT_sb = pp.tile([P, 2, T], F32R)    # attention out^T (normalized)

            # ones columns of v_sb (softmax denominator accumulator)
            ones64 = cp.tile([P, NT * HL], F32)
            nc.vector.memset(ones64[:], 1.0)
            vones = v_sb.rearrange("p n (h e) -> p n h e", h=HL)[:, :, :, D:D + 1]
            nc.vector.tensor_copy(vones, ones64[:].rearrange(
                "p (n h) -> p n h", n=NT)[:, :, :, None])

            with tc.tile_pool(name="xw", bufs=1) as xw:
                xTr = xw.tile([P, KC, T], F32R)
                for kk in range(KC):
                    nc.sync.dma_start(
                        xTr[:, kk, :],
                        xT.rearrange("(k p) t -> k p t", p=P)[kk])
                wq_sb = xw.tile([P, KC, DL], F32R)
                wk_sb = xw.tile([P, KC, DL], F32R)
                wv_sb = xw.tile([P, KC, DL], F32R)
                nc.sync.dma_start(wq_sb[:], wq.rearrange("(k p) m -> p k m", p=P))
                nc.sync.dma_start(wk_sb[:], wk.rearrange("(k p) m -> p k m", p=P))
                nc.sync.dma_start(wv_sb[:], wv.rearrange("(k p) m -> p k m", p=P))

                # ---- phase A: qkv projection ----
                for w_sb, b_col, dst in ((wq_sb, bq_col, qT_sb),
                                         (wk_sb, bk_col, kT_sb)):
                    for mi in range(2):
                        for ni in range(NQ):
                            ps = ps_proj.tile([P, QB], F32, name="proj_ps",
                                              tag="proj_ps")
                            for kk in range(KC):
                                nc.tensor.matmul(
                                    ps[:],
                                    w_sb[:, kk, mi * P:(mi + 1) * P],
                                    xTr[:, kk, ni * QB:(ni + 1) * QB],
                                    start=(kk == 0), stop=(kk == KC - 1))
                            nc.vector.tensor_scalar_add(
                                dst[:, mi, ni * QB:(ni + 1) * QB], ps[:],
                                b_col[:, mi:mi + 1])
                for ti in range(NT):
                    ps = ps_proj.tile([P, DL], F32, name="proj_ps",
                                      tag="proj_ps")
                    for kk in range(KC):
                        nc.tensor.matmul(ps[:], xTr[:, kk, ti * P:(ti + 1) * P],
                                         wv_sb[:, kk, :],
                                         start=(kk == 0), stop=False)
                    nc.tensor.matmul(ps[:], ones_r[:, :P], bv_sb[:],
                                     start=False, stop=True)
                    nc.vector.tensor_copy(
                        v_sb.rearrange("p n (h e) -> p n h e", h=HL)
                        [:, ti, :, 0:D],
                        ps[:].rearrange("p (h e) -> p h e", e=D))
            # xw pool released; wo loads overlap attention below.

            with tc.tile_pool(name="wo_pool", bufs=1) as wop:
                wo_sb = wop.tile([P, 2, C], F32R)
                nc.sync.dma_start(wo_sb[:],
                                  wo.rearrange("(k p) n -> p k n", p=P))
                bo_bc = wop.tile([P, C], F32)
                nc.gpsimd.partition_broadcast(bo_bc[:], bo_sb[:].bitcast(F32))

                # ---- phases B+C interleaved per query chunk ----
                BF16 = mybir.dt.bfloat16
                part_dram = dp.tile([T, C], BF16)
                rs_out = dp.tile([NQ, P, C], BF16)
                for qc in range(NQ):
                    # flash attention: head pairs interleaved at the
                    # key-block level so PE stays busy while ACT runs exp
                    nkb = 4 * qc + 4
                    for hp in range(HL // 2):
                        heads = (2 * hp, 2 * hp + 1)
                        pvs = {}
                        for h in heads:
                            pvs[h] = ps_pv.tile([P, QB], F32,
                                                name="pv_ps", tag="pv")
                        # full (unmasked) key-block pairs
                        for kp in range(2 * qc):
                            for h in heads:
                                po = 64 * (h % 2)
                                mi = h // 2
                                sT = ps_sT.tile([P, 2 * QB], F32,
                                                name="sT_ps", tag="sT")
                                for half in range(2):
                                    kb = 2 * kp + half
                                    nc.tensor.matmul(
                                        sT[:, half * QB:(half + 1) * QB],
                                        kT_sb[po:po + D, mi,
                                              kb * P:(kb + 1) * P],
                                        qT_sb[po:po + D, mi,
                                              qc * QB:(qc + 1) * QB],
                                        start=True, stop=True)
                                pT = wk_p.tile([P, 2 * QB], F32R, name="pT",
                                               tag="pT")
                                nc.scalar.activation(
                                    pT[:], sT[:],
                                    mybir.ActivationFunctionType.Exp,
                                    scale=SCALE)
                                for half in range(2):
                                    kb = 2 * kp + half
                                    nc.tensor.matmul(
                                        pvs[h][0:D + 1, :],
                                        v_sb[:, kb,
                                             h * (D + 1):(h + 1) * (D + 1)],
                                        pT[:, half * QB:(half + 1) * QB],
                                        start=(kb == 0), stop=False)
                        # diagonal blocks, queries narrowed to the visible
                        # range [128*di, 512); only a [128,128] tril masked
                        for di in range(4):
                            kb = 4 * qc + di
                            q0 = di * P          # first visible query col
                            qw = QB - q0
                            for h in heads:
                                po = 64 * (h % 2)
                                mi = h // 2
                                sT = ps_sT.tile([P, 2 * QB], F32,
                                                name="sT_ps", tag="sT")
                                nc.tensor.matmul(
                                    sT[:, 0:qw],
                                    kT_sb[po:po + D, mi,
                                          kb * P:(kb + 1) * P],
                                    qT_sb[po:po + D, mi,
                                          qc * QB + q0:(qc + 1) * QB],
                                    start=True, stop=True)
                                nc.vector.tensor_add(
                                    sT[:, 0:P], sT[:, 0:P], masks[:])
                                pT = wk_p.tile([P, 2 * QB], F32R, name="pT",
                                               tag="pT")
                                nc.scalar.activation(
                                    pT[:, 0:qw], sT[:, 0:qw],
                                    mybir.ActivationFunctionType.Exp,
                                    scale=SCALE)
                                nc.tensor.matmul(
                                    pvs[h][0:D + 1, q0:QB],
                                    v_sb[:, kb,
                                         h * (D + 1):(h + 1) * (D + 1)],
                                    pT[:, 0:qw],
                                    start=(qc == 0 and di == 0),
                                    stop=(di == 3))
                        for h in heads:
                            po = 64 * (h % 2)
                            mi = h // 2
                            rbc = wk_p.tile([D, QB], F32, name="rbc",
                                            tag="rbc", bufs=2)
                            lrow = wk_p.tile([1, QB], F32, name="lrow",
                                             tag="lrow", bufs=2)
                            nc.scalar.copy(lrow[:], pvs[h][D:D + 1, :])
                            nc.vector.reciprocal_approx_fast(
                                out=rbc[0:1, :], in_=lrow[:])
                            nc.gpsimd.partition_broadcast(rbc[:], rbc[0:1, :])
                            nc.vector.tensor_mul(
                                aoT_sb[po:po + D, mi, qc * QB:(qc + 1) * QB],
                                pvs[h][0:D, :], rbc[:])

                    # partial output projection for this chunk's tokens
                    for mi2 in range(4 * qc, 4 * qc + 4):
                        for ni in range(2):
                            ps = ps_proj.tile([P, QB], F32, name="proj_ps",
                                              tag="proj_ps")
                            for kk in range(2):
                                nc.tensor.matmul(
                                    ps[:],
                                    aoT_sb[:, kk, mi2 * P:(mi2 + 1) * P],
                                    wo_sb[:, kk, ni * QB:(ni + 1) * QB],
                                    start=(kk == 0), stop=(kk == 1))
                            o_sb = wk_p.tile([P, QB], BF16, name="o_sb",
                                             tag="o_sb", bufs=2)
                            nc.scalar.copy(o_sb[:], ps[:])
                            nc.sync.dma_start(
                                part_dram[mi2 * P:(mi2 + 1) * P,
                                          ni * QB:(ni + 1) * QB],
                                o_sb[:])

                    # chunked ReduceScatter overlaps later chunks' attention
                    nc.gpsimd.collective_compute(
                        "ReduceScatter",
                        mybir.AluOpType.add,
                        replica_groups=[[0, 1, 2, 3], [4, 5, 6, 7]],
                        ins=[part_dram[qc * QB:(qc + 1) * QB, :]],
                        outs=[rs_out[qc]],
                    )
                    r_sb = wk_p.tile([P, C], BF16, name="r_sb", tag="r_sb",
                                     bufs=2)
                    nc.sync.dma_start(r_sb[:], rs_out[qc])
                    f_sb = wk_p.tile([P, C], F32, name="f_sb", tag="f_sb",
                                     bufs=2)
                    nc.vector.tensor_add(f_sb[:], r_sb[:], bo_bc[:])
                    nc.sync.dma_start(out[qc], f_sb[:])

    nc.compile()
    return nc


def kernel(x, w_qkv, b_qkv, w_out, b_out):
    x = np.ascontiguousarray(np.asarray(x, dtype=np.float32))
    w_qkv = np.asarray(w_qkv, dtype=np.float32)
    b_qkv = np.asarray(b_qkv, dtype=np.float32)
    w_out = np.ascontiguousarray(np.asarray(w_out, dtype=np.float32))
    b_out = np.asarray(b_out, dtype=np.float32)

    if "nc" not in _CACHED:
        _CACHED["nc"] = _build()
    nc = _CACHED["nc"]

    xTs = [np.ascontiguousarray(x[b_].T) for b_ in range(B)]
    bo = np.ascontiguousarray(b_out[None, :])
    in_maps = []
    for c in range(8):
        b_, g = c // 4, c % 4
        sl = slice(g * DL, (g + 1) * DL)
        in_maps.append({
            "xT": xTs[b_],
            "wq": np.ascontiguousarray(w_qkv[:, 0 * C:1 * C][:, sl]),
            "wk": np.ascontiguousarray(w_qkv[:, 1 * C:2 * C][:, sl]),
            "wv": np.ascontiguousarray(w_qkv[:, 2 * C:3 * C][:, sl]),
            "bq": np.ascontiguousarray(b_qkv[0 * C:1 * C][sl][None, :]),
            "bk": np.ascontiguousarray(b_qkv[1 * C:2 * C][sl][None, :]),
            "bv": np.ascontiguousarray(b_qkv[2 * C:3 * C][sl][None, :]),
            "wo": np.ascontiguousarray(w_out[g * DL:(g + 1) * DL, :]),
            "bo": bo,
        })
    res = run_bass_kernel_spmd(nc, in_maps, list(range(8)))
    out_full = np.empty((B, T, C), dtype=np.float32)
    for c in range(8):
        b_, g = c // 4, c % 4
        o = res.results[c]["out"]          # [NQ, P, C]
        for qc in range(NQ):
            r0 = qc * QB + g * P
            out_full[b_, r0:r0 + P, :] = o[qc]
    return out_full
